# revision 97
# baseline (speedup 1.0000x reference)
"""Trainium2 Bass kernel for a GQA attention layer (dense transformer).

Reference computation (B=1, S=2048, DIM=2048, 32 q-heads, 8 kv-heads, hd=64):
    xq = x @ wq; xk = x @ wk; xv = x @ wv
    rope(xq, xk); GQA causal attention; out = attn @ wo

Sharding: tensor-parallel over heads across 8 cores. Core c owns q-heads
4c..4c+3 (wq cols), kv-head c (wk/wv cols), and wo rows 256c..256c+255.
Each core computes a full [S, DIM] partial of the output projection; the
host sums the 8 partials (the TP all-reduce, done at gather time).

Kernel layout strategy (everything "transposed", head_dim on partitions):
  - QT/KT/VT = W.T @ x as compensated-fp8 DoubleRow matmuls: x and wqkv are
    split hi/lo into e4m3 on the host (power-of-two scales SX/SW, folded
    back out through cos/sin for Q/K and the V-copy scalar). Each DR matmul
    contracts a pair of 128-row k-tiles at 0.5 cycles/column; 3 products
    (hi*hi + lo*hi + hi*lo) give better-than-bf16 accuracy at 75% of the
    bf16 PE cost.
  - RoPE pairs are de-interleaved by permuting wq/wk columns on the host so
    the rotation partner sits 16 partitions away (within a 32-partition
    quadrant, so DVE stream_shuffle can swap them); bf16 temporaries keep
    most rope ops in the DVE fast modes.
  - Scores are computed transposed: S^T[k, q] = K^T.T @ Q^T per 128-row
    k-tile; exp on ACT (scale fused); causal mask = upper-tri multiply on
    the single diagonal 128x128 block of each k-tile.
  - P@V is computed as V'.T @ P^T where V' = [V | ones]: the ones column
    makes row 64 of the PSUM accumulator the softmax denominator for free.
  - PV runs two heads behind scores (deep software pipeline) so the
    exp->mask latency of head h hides under PV of head h-2.
  - Normalization: reciprocal (DVE) + partition_broadcast (GPSIMD) + mult,
    then a hi/lo e4m3 split of O (on the idle GPSIMD engine; DVE for the
    last chunk) feeding a compensated-fp8 DoubleRow output projection
    (wo is host-split hi/lo; scales folded out in the PSUM->SBUF copy).
  - WO partials DMA'd bf16 and summed on the host (the TP all-reduce).
"""

import numpy as np
import ml_dtypes

import concourse.bass as bass
import concourse.mybir as mybir
from concourse import bacc
from concourse.tile import TileContext
from concourse.masks import make_identity, make_upper_triangular
from concourse.bass_utils import run_bass_kernel_spmd

# ---------------------------------------------------------------- constants
S = 2048          # sequence length
DIM = 2048        # model dim
NH = 32           # query heads
NKV = 8           # kv heads
HD = 64           # head dim
NCORES = 8
HQ = NH // NCORES          # query heads per core = 4
QW = HQ * HD               # q width per core = 256
KT_S = S // 128            # 16 seq k-tiles
KT_D = DIM // 128          # 16 dim k-tiles
NSC = S // 512             # 4 s-chunks
SCALE = 1.0 / 8.0          # 1/sqrt(64)

# fp8 hi/lo split scales for the projections (powers of two; folded back out
# via cos/sin for Q/K and the V-copy scalar). x absmax ~5, w absmax ~0.11.
SX = 32.0
SW = 1024.0
INV_SXW = 1.0 / (SX * SW)
# fp8 scales for the output projection (O is rescaled at the V-copy; the
# product scale is folded out in the PSUM->SBUF copy of the WO result)
SO = 32.0
SWO = 1024.0
INV_OW = 1.0 / (SO * SWO)

# matmul dtype knob: 'bf16' | 'f32' | 'f32r'
MM = 'bf16'

_SHUF_SWAP16 = list(range(16, 32)) + list(range(16))


def _dtypes():
    if MM == 'bf16':
        return mybir.dt.bfloat16, mybir.dt.bfloat16, ml_dtypes.bfloat16
    if MM == 'f32':
        return mybir.dt.float32, mybir.dt.float32, np.float32
    if MM == 'f32r':
        return mybir.dt.float32, mybir.dt.float32r, np.float32
    raise ValueError(MM)


def _mm_ap(ap, mmdt):
    """View an AP in the matmul dtype (bitcast f32 -> f32r when needed)."""
    if ap.dtype != mmdt:
        return ap.bitcast(mmdt)
    return ap


def build_program():
    """Build the per-core Bass program (same program on all 8 cores).

    Emission is a fine-grained software pipeline: attention beats for chunk
    sc (S^T mega-matmul for head h + PV pairs of head h-1) are merged with
    the projection matmuls of chunk sc+1 and the WO units of chunk sc-1.
    Engines execute in order, so the merge keeps only likely-ready work in
    the PE stream while ACT (exp) and DVE (RoPE/normalize) drain.

    PSUM (8 banks): pjo 3 (projection passes + PV accumulators, shared tag)
    + ps 4 (two [128,1024] score megas) + pw 1 (WO).
    """
    sdt, mmdt, _ = _dtypes()
    f32 = mybir.dt.float32

    nc = bacc.Bacc("TRN2", target_bir_lowering=False, debug=False,
                   num_devices=NCORES)

    f8 = mybir.dt.float8e4
    DR = mybir.MatmulPerfMode.DoubleRow
    xT = nc.dram_tensor("xT8", [DIM, S, 2], f8, kind="ExternalInput")
    wqkv = nc.dram_tensor("wqkv8", [DIM, QW + 2 * HD, 2], f8,
                          kind="ExternalInput")
    wo_s = nc.dram_tensor("wo8", [QW, DIM, 2], f8, kind="ExternalInput")
    cosE = nc.dram_tensor("cosE", [64, S], sdt, kind="ExternalInput")
    sinE = nc.dram_tensor("sinE", [64, S], sdt, kind="ExternalInput")
    out = nc.dram_tensor("out", [S, DIM], sdt, kind="ExternalOutput")

    WQKV = QW + 2 * HD  # 384

    import contextlib
    with TileContext(nc) as tc, contextlib.ExitStack() as ctx:
        const = ctx.enter_context(tc.tile_pool(name="const", bufs=1))
        work = ctx.enter_context(tc.tile_pool(name="work", bufs=2))
        xtp = ctx.enter_context(tc.tile_pool(name="xtp", bufs=9))
        ptp = ctx.enter_context(tc.tile_pool(name="ptp", bufs=26))
        small = ctx.enter_context(tc.tile_pool(name="small", bufs=5))
        osb = ctx.enter_context(tc.tile_pool(name="osb", bufs=4))

        pjo = ctx.enter_context(tc.tile_pool(name="pjo", bufs=3,
                                             space="PSUM"))
        ps = ctx.enter_context(tc.tile_pool(name="ps", bufs=2, space="PSUM"))
        pw = ctx.enter_context(tc.tile_pool(name="pw", bufs=1, space="PSUM"))

        # ----------------------------------------------- persistent SBUF
        w_sb = const.tile([128, KT_D * 2 * WQKV], f8, tag="w_sb")
        wo_sb = const.tile([128, 2 * DIM * 2], f8, tag="wo_sb")
        cos_sb = const.tile([128, S], sdt, tag="cos_sb")
        sin_sb = const.tile([128, S], sdt, tag="sin_sb")
        utri_sb = const.tile([128, 128], sdt, tag="utri_sb")
        ident = const.tile([128, 128], sdt, tag="ident")
        QT = const.tile([64, HQ * S], sdt, tag="QT")
        KVt = const.tile([128, S], sdt, tag="KVt")
        Vp = const.tile([128, KT_S * (HD + 1)], sdt, tag="Vp")
        OTh = const.tile([128, 2 * S], f8, tag="OTh")
        OTl = const.tile([128, 2 * S], f8, tag="OTl")

        # Warm-ups FIRST (minimal dependencies): a throwaway matmul starts
        # the PE p-state ramp clock at t~0 (so the real prologue runs at
        # full clock), and a throwaway exp pulls the ACT table load off the
        # first score tile's critical path.
        warm_src = small.tile([128, 16], sdt, tag="warm_src",
                              name="warm_src")
        warm_ps = pw.tile([16, 16], f32, tag="pw", name="warm")
        warm_sb = small.tile([1, 1], f32, tag="warm_sb", name="warm_sb")
        nc.gpsimd.memset(warm_src[:], 1.0)
        nc.tensor.matmul(warm_ps[:], warm_src[:], warm_src[:],
                         start=True, stop=True)
        nc.scalar.activation(warm_sb[:], warm_ps[0:1, 0:1],
                             mybir.ActivationFunctionType.Exp)

        make_identity(nc, ident[:])
        make_upper_triangular(nc, utri_sb[:], val=1.0)
        nc.gpsimd.memset(Vp[:], 1.0)  # ones columns for denominator

        wo_copy_flip = [0]

        # w_sb layout: [128, kt, hl, WQKV]; xt tiles: [128, kt(4), hl, 512].
        # Projections run as compensated fp8 in DoubleRow mode: each matmul
        # contracts a PAIR of adjacent 128-row k-tiles (one hi/lo plane) at
        # 0.5 cycles/column, and each k-pair needs 3 products
        # (hi*hi + lo*hi + hi*lo) -- 24 matmuls per 16-k-tile pass vs 16
        # full-rate bf16 matmuls, a 25% PE saving at better-than-bf16
        # accuracy.
        wv_ = w_sb[:].rearrange("r (k w t) -> r k w t", k=KT_D, t=2)
        NPAIR = KT_D // 2
        PRODS = ((0, 0), (1, 0), (0, 1))  # (w-plane, x-plane)

        # ---------------------------------------------- thunk generators
        def proj_thunks(sc):
            """Projection of chunk sc, unfused: KV pass over all 8 k-pairs
            (one PSUM slot), then K-rope + V copy/transposes overlapped with
            the two Q passes."""
            s0 = sc * 512
            xts = []
            st = {}

            batches = [2, 2, 4, 4, 4] if sc == 0 else [4, 4, 4, 4]
            starts = [sum(batches[:i]) for i in range(len(batches))]
            pair_slot = {}
            for bi, (b0, bn) in enumerate(zip(starts, batches)):
                for j in range(0, bn, 2):
                    pair_slot[(b0 + j) // 2] = (bi, j, bn, b0)

            def xtv(a):
                bi, j, bn, b0 = pair_slot[a]
                return xts[bi][:].rearrange(
                    "r (k c t) -> r k c t", k=4, t=2)[:, j:j + 2]

            def dma_pair(a):
                bi, j, bn, b0 = pair_slot[a]
                if j == 0:
                    if sc == 0:
                        nc.sync.dma_start(
                            w_sb[:, b0 * 2 * WQKV:(b0 + bn) * 2 * WQKV
                                 ].rearrange("r (k w) -> r k w", k=bn),
                            wqkv[b0 * 128:(b0 + bn) * 128].rearrange(
                                "(k r) w t -> r k (w t)", k=bn))
                    xt4 = xtp.tile([128, 4 * 2 * 512], f8, tag="xt",
                                   name="xt4")
                    nc.sync.dma_start(
                        xt4[:, 0:bn * 1024].rearrange(
                            "r (k c) -> r k c", k=bn),
                        xT[b0 * 128:(b0 + bn) * 128,
                           s0:s0 + 512].rearrange(
                            "(k r) c t -> r k (c t)", k=bn))
                    xts.append(xt4)
                    if sc == 0 and bi == 2:
                        # constants ride behind the first x/w batches but
                        # before the last ones: they are needed by q_rope
                        # right after the first pass stops
                        # (cos/sin have 64-row periodicity: DMA'd twice)
                        nc.sync.dma_start(cos_sb[0:64, :], cosE[:])
                        nc.sync.dma_start(sin_sb[0:64, :], sinE[:])
                        nc.sync.dma_start(cos_sb[64:128, :], cosE[:])
                        nc.sync.dma_start(sin_sb[64:128, :], sinE[:])

            def kv_mm(a):
                if a == 0:
                    st["pkv"] = pjo.tile([128, 512], f32, tag="pjo",
                                         name="pkv")
                xv = xtv(a)
                for pi, (wp, xp) in enumerate(PRODS):
                    nc.tensor.matmul(
                        st["pkv"][:],
                        wv_[:, 2 * a:2 * a + 2, 256:384, wp],
                        xv[:, :, :, xp],
                        start=(a == 0 and pi == 0),
                        stop=(a == NPAIR - 1 and pi == len(PRODS) - 1),
                        perf_mode=DR)

            def k_rope():
                pkv = st["pkv"]
                shufk = work.tile([64, 512], f32, tag="shufk", name="shufk")
                m1k = work.tile([64, 512], sdt, tag="m1k", name="m1k")
                t2k = work.tile([64, 512], sdt, tag="t2k", name="t2k")
                nc.vector.stream_shuffle(shufk[:], pkv[0:64, :],
                                         _SHUF_SWAP16)
                nc.vector.tensor_mul(m1k[:], pkv[0:64, :],
                                     cos_sb[0:64, s0:s0 + 512])
                nc.vector.tensor_mul(t2k[:], shufk[:],
                                     sin_sb[0:64, s0:s0 + 512])
                nc.vector.tensor_add(KVt[0:64, s0:s0 + 512], m1k[:], t2k[:])

            def v_copy():
                # also folds the fp8 scale back out of V
                nc.vector.tensor_scalar_mul(KVt[64:128, s0:s0 + 512],
                                            st["pkv"][64:128, :],
                                            INV_SXW * SO)

            def v_trans():
                # all 4 transposes of the chunk go into one PSUM tile as a
                # single accumulation group (disjoint columns), then one
                # strided ACT copy drops them into Vp (ACT is idle here)
                pv4 = pw.tile([128, 4 * HD], sdt, tag="pw", name="pv4")
                for j in range(4):
                    kt = 4 * sc + j
                    nc.tensor.matmul(
                        pv4[:, j * HD:(j + 1) * HD],
                        KVt[64:128, kt * 128:(kt + 1) * 128],
                        ident[64:128, 64:128],
                        is_transpose=True, start=(j == 0), stop=(j == 3))
                dst = Vp[:, 4 * sc * (HD + 1):(4 * sc + 4) * (HD + 1)]
                nc.scalar.copy(
                    dst.rearrange("r (k w) -> r k w", k=4)[:, :, 0:HD],
                    pv4[:].rearrange("r (k w) -> r k w", k=4))

            def q_mm(mt, a):
                if mt == 0:
                    dma_pair(a)
                if a == 0:
                    st["pq"] = pjo.tile([128, 512], f32, tag="pjo",
                                        name="pq")
                xv = xtv(a)
                for pi, (wp, xp) in enumerate(PRODS):
                    nc.tensor.matmul(
                        st["pq"][:],
                        wv_[:, 2 * a:2 * a + 2,
                            mt * 128:mt * 128 + 128, wp],
                        xv[:, :, :, xp],
                        start=(a == 0 and pi == 0),
                        stop=(a == NPAIR - 1 and pi == len(PRODS) - 1),
                        perf_mode=DR)

            def q_rope(mt):
                pq = st["pq"]
                shuf = work.tile([128, 512], f32, tag="shuf", name="shuf")
                m1 = work.tile([128, 512], sdt, tag="m1", name="m1")
                t2 = work.tile([128, 512], sdt, tag="t2", name="t2")
                nc.vector.stream_shuffle(shuf[:], pq[:], _SHUF_SWAP16)
                nc.vector.tensor_mul(m1[:], pq[:], cos_sb[:, s0:s0 + 512])
                nc.vector.tensor_mul(t2[:], shuf[:],
                                     sin_sb[:, s0:s0 + 512])
                he = (2 * mt) * S
                ho = (2 * mt + 1) * S
                nc.vector.tensor_add(
                    QT[:, he + s0:he + s0 + 512], m1[0:64, :], t2[0:64, :])
                nc.vector.tensor_add(
                    QT[:, ho + s0:ho + s0 + 512], m1[64:128, :],
                    t2[64:128, :])

            # pass order Q0 -> KV -> Q1: the Q0 rope chain (longest,
            # and first consumed by scores) starts as soon as the last x
            # batch lands, overlapping the KV and Q1 passes on PE; K-rope
            # and the V path overlap Q1.
            th = [lambda a=a: q_mm(0, a) for a in range(NPAIR)]
            th.append(lambda: q_rope(0))
            th += [lambda a=a: kv_mm(a) for a in range(NPAIR)]
            th.append(v_copy)
            th.append(k_rope)
            qth = [lambda a=a: q_mm(1, a) for a in range(NPAIR)]
            qth.insert(2, v_trans)
            th += qth
            th.append(lambda: q_rope(1))
            return th

        def s_thunks(qc, h, tiles):
            """S^T mega matmuls + exp + mask for one head; fills `tiles`."""
            q0 = qc * 512
            hf = h * S
            nkt = 4 * qc + 4
            thunks = []
            for pi in range(nkt // 2):
                def th(pi=pi):
                    kts = (2 * pi, 2 * pi + 1)
                    ps_t = ps.tile([128, 1024], f32, tag="ps", name="ps_t")
                    pt_t = ptp.tile([128, 1024], sdt, tag="pt", name="pt_t")
                    for li, kt in enumerate(kts):
                        dj = kt - 4 * qc
                        qo = 128 * dj if dj >= 0 else 0
                        lo = li * 512
                        nc.tensor.matmul(
                            ps_t[:, lo + qo:lo + 512],
                            _mm_ap(KVt[0:64, kt * 128:(kt + 1) * 128], mmdt),
                            _mm_ap(QT[:, hf + q0 + qo:hf + q0 + 512], mmdt),
                            start=True, stop=True)
                    if 2 * pi + 1 < 4 * qc:
                        nc.scalar.activation(
                            pt_t[:], ps_t[:],
                            mybir.ActivationFunctionType.Exp, scale=SCALE)
                    else:
                        for li, kt in enumerate(kts):
                            dj = kt - 4 * qc
                            qo = 128 * dj if dj >= 0 else 0
                            lo = li * 512
                            nc.scalar.activation(
                                pt_t[:, lo + qo:lo + 512],
                                ps_t[:, lo + qo:lo + 512],
                                mybir.ActivationFunctionType.Exp,
                                scale=SCALE)
                    for li, kt in enumerate(kts):
                        dj = kt - 4 * qc
                        qo = 128 * dj if dj >= 0 else 0
                        lo = li * 512
                        if dj >= 0:
                            nc.vector.tensor_mul(
                                pt_t[:, lo + qo:lo + qo + 128],
                                pt_t[:, lo + qo:lo + qo + 128], utri_sb[:])
                        tiles.append((kt, qo, lo, pt_t))
                thunks.append(th)
            return thunks

        def pv_thunks(qc, h, tiles):
            """PV accumulation pairs + final normalization for one head."""
            q0 = qc * 512
            hp = (h % 2) * 64
            nkt0 = 4 * qc + 4
            state = {}

            def pv_pair(pi):
                if "po" not in state:
                    state["po"] = pjo.tile([HD + 1, 512], f32, tag="pjo",
                                           name="pot")
                po_t = state["po"]
                for kt, qo, lo, pt_t in tiles[2 * pi:2 * pi + 2]:
                    nc.tensor.matmul(
                        po_t[:, qo:512],
                        _mm_ap(Vp[:, kt * (HD + 1):(kt + 1) * (HD + 1)],
                               mmdt),
                        _mm_ap(pt_t[:, lo + qo:lo + 512], mmdt),
                        start=(kt == 0), stop=(kt == nkt0 - 1))
                if 2 * pi + 2 >= nkt0:
                    rc = small.tile([1, 512], f32, tag="rc", name="rc")
                    rb = small.tile([64, 512], f32, tag="rb", name="rb")
                    tn = work.tile([128, 512], sdt, tag="tn",
                                   name="tn")[hp:hp + 64, :]
                    nc.vector.reciprocal(rc[:], po_t[64:65, :])
                    nc.gpsimd.partition_broadcast(rb[:], rc[:])
                    of = (h // 2) * S
                    sl = slice(of + q0, of + q0 + 512)
                    nc.vector.tensor_mul(tn[:], po_t[0:64, :], rb[:])
                    # hi/lo fp8 split of O: on the idle GPSIMD engine for
                    # chunks 0-2; on DVE for the last chunk (shorter tail)
                    eng = nc.vector if qc == NSC - 1 else nc.gpsimd
                    eng.tensor_copy(OTh[hp:hp + 64, sl], tn[:])
                    eng.tensor_sub(OTl[hp:hp + 64, sl], tn[:],
                                   OTh[hp:hp + 64, sl])

            return [lambda pi=pi: pv_pair(pi) for pi in range(nkt0 // 2)]

        def wo_half(qt, np2, half, obs, pool=None, ptag="pw",
                    act_copy=False, dma_mode="row"):
            """One 512-wide n-chunk. dma_mode picks the output-DMA grain:
            "row" = one [128,2048] DMA per q-tile (fewest descriptors, for
            the steady state), "pair" = [128,1024] per np2 pair, "split" =
            [128,512] per chunk (for the very last q-tile so only 512
            columns ride the drain)."""
            pool = pool or pw
            ncn = 2 * np2 + half
            if ncn == 0:
                obs[qt] = osb.tile([128, 2048], sdt, tag="ob", name="ob")
            ob = obs[qt]
            pw_t = pool.tile([128, 512], f32, tag=ptag, name="pw_t")
            qs = slice(qt * 128, qt * 128 + 128)
            ds = slice(ncn * 512, ncn * 512 + 512)
            OThv = OTh[:].rearrange("r (m s) -> r m s", m=2)
            OTlv = OTl[:].rearrange("r (m s) -> r m s", m=2)
            wov = wo_sb[:].rearrange("r (m d t) -> r m d t", m=2, t=2)
            for pi, (o_, w_) in enumerate(((OThv, 0), (OThv, 1), (OTlv, 0))):
                nc.tensor.matmul(
                    pw_t[:], o_[:, :, qs], wov[:, :, ds, w_],
                    start=(pi == 0), stop=(pi == 2), perf_mode=DR)
            if act_copy:
                nc.scalar.mul(ob[:, ncn * 512:ncn * 512 + 512], pw_t[:],
                              INV_OW)
            else:
                nc.vector.tensor_scalar_mul(
                    ob[:, ncn * 512:ncn * 512 + 512], pw_t[:], INV_OW)
            if dma_mode == "split":
                nc.sync.dma_start(
                    out[qt * 128:(qt + 1) * 128, ncn * 512:ncn * 512 + 512],
                    ob[:, ncn * 512:ncn * 512 + 512])
            elif dma_mode == "pair" and half == 1:
                nc.sync.dma_start(
                    out[qt * 128:(qt + 1) * 128,
                        np2 * 1024:np2 * 1024 + 1024],
                    ob[:, np2 * 1024:np2 * 1024 + 1024])
            elif dma_mode == "row" and ncn == 3:
                nc.sync.dma_start(out[qt * 128:(qt + 1) * 128, :], ob[:])
            if ncn == 3:
                del obs[qt]
                wo_copy_flip[0] += 1
        wo_obs = {}

        # ------------------------------------- merged emission schedule
        def merge(primary, *others):
            """Emit primary thunks; proportionally interleave the others."""
            counters = [0.0] * len(others)
            n = max(1, len(primary))
            for beat in primary:
                for j, lst in enumerate(others):
                    counters[j] += len(lst) / n
                    while counters[j] >= 1.0 and lst:
                        lst.pop(0)()
                        counters[j] -= 1.0
                for th in beat:
                    th()
            for lst in others:
                while lst:
                    lst.pop(0)()

        for th in proj_thunks(0):                   # prologue
            th()

        hist = []                        # (qc, h, tiles) awaiting PV
        for sc in range(NSC):
            if sc == 1:
                nc.sync.dma_start(
                    wo_sb[:, 0:2 * DIM],
                    wo_s[0:128].rearrange("r d t -> r (d t)"))
                nc.sync.dma_start(
                    wo_sb[:, 2 * DIM:4 * DIM],
                    wo_s[128:256].rearrange("r d t -> r (d t)"))
            pstream = proj_thunks(sc + 1) if sc + 1 < NSC else []
            wostream = ([lambda qt=qt, np2=np2, half=half, ac=False:
                         wo_half(qt, np2, half, wo_obs, act_copy=ac)
                         for qt in range(4 * (sc - 1), 4 * (sc - 1) + 4)
                         for np2 in range(2)
                         for half in range(2)] if sc >= 1 else [])
            for h in range(HQ):
                tiles = []
                sth = s_thunks(sc, h, tiles)
                pth = pv_thunks(*hist[-2]) if len(hist) >= 2 else []
                beats = []
                for i in range(max(len(sth), len(pth))):
                    beat = []
                    if i < len(pth):
                        beat.append(pth[i])
                    if i < len(sth):
                        beat.append(sth[i])
                    beats.append(beat)
                # WO of sc-1 needs PV(sc-1, 3) done: that PV is head 0 here
                if h == 0:
                    ptake = max(1, len(pstream) // HQ) if pstream else 0
                    merge(beats, pstream[:ptake])
                    pstream = pstream[ptake:]
                else:
                    # on the last chunk hold 4 WO thunks back: they fill the
                    # PE while the final head's normalize chain drains
                    hold = 3 if sc == NSC - 1 else 0
                    avail = max(0, len(wostream) - hold)
                    ptake = (len(pstream) // (HQ - h)) if pstream else 0
                    wtake = (avail // (HQ - h)) if avail and h >= 3 else 0
                    merge(beats, pstream[:ptake], wostream[:wtake])
                    pstream = pstream[ptake:]
                    wostream = wostream[wtake:]
                hist.append((sc, h, tiles))
            if sc < NSC - 1:
                merge([], pstream, wostream)

        # epilogue: PV of the last head (+ the held-back chunk-2 WO units
        # covering its normalize), then WO of chunk 3; the score pool's
        # banks are free now, so WO rotates through those too
        for th in pv_thunks(*hist[-2]):
            th()
        for i, th in enumerate(wostream):
            th(ac=True)
            if i == 1:
                for t2 in pv_thunks(*hist[-1]):
                    t2()
        epi = 0
        pools = [(pw, "pw"), (ps, "ps"), (pjo, "pjo"),
                 (ps, "ps"), (pjo, "pjo"), (pjo, "pjo")]
        for qt in (12, 13, 14, 15):
            for np2 in range(2):
                for half in range(2):
                    pool, ptag = pools[epi % len(pools)]
                    wo_half(qt, np2, half, wo_obs, pool=pool, ptag=ptag,
                            act_copy=(epi % 2 == 0),
                            dma_mode="pair")
                    epi += 1

    nc.compile()
    return nc


# ------------------------------------------------------------- host side
def _pair_perm64():
    """Column permutation putting the RoPE partner 16 partitions away."""
    return np.array([2 * (16 * (j // 32) + (j % 16)) + ((j % 32) // 16)
                     for j in range(64)])


def _host_prep(x, freqs_cos, freqs_sin, wq, wk, wv, wo):
    _, _, npdt = _dtypes()
    x = np.asarray(x, np.float32)
    fc = np.asarray(freqs_cos, np.float32)
    fs = np.asarray(freqs_sin, np.float32)
    wq = np.asarray(wq, np.float32)
    wk = np.asarray(wk, np.float32)
    wv = np.asarray(wv, np.float32)
    wo = np.asarray(wo, np.float32)

    perm = _pair_perm64()
    f8np = ml_dtypes.float8_e4m3

    def split8(a, s):
        hi = np.asarray(a * s, f8np)
        lo = np.asarray(a * s - hi.astype(np.float32), f8np)
        return np.ascontiguousarray(np.stack([hi, lo], axis=-1))

    xT8 = split8(x[0].T, SX)                 # [DIM, 2, S] fp8

    p = np.arange(64)
    pair = 16 * ((p % 64) // 32) + (p % 16)
    sign = np.where((p % 32) < 16, -1.0, 1.0).astype(np.float32)
    # cos/sin also fold the fp8 projection scale back out of Q and K
    cosE = np.ascontiguousarray(fc[:, pair].T) * INV_SXW        # [64, S]
    sinE = (np.ascontiguousarray(fs[:, pair].T) * sign[:, None]) * INV_SXW

    in_maps = []
    for c in range(NCORES):
        qcols = np.concatenate(
            [wq[:, (4 * c + i) * 64 + perm] for i in range(HQ)], axis=1)
        kcols = wk[:, c * 64 + perm]
        vcols = wv[:, c * 64:(c + 1) * 64]
        wqkv_c = np.concatenate([qcols, kcols, vcols], axis=1)
        in_maps.append({
            "xT8": xT8,
            "wqkv8": split8(wqkv_c, SW),
            "wo8": split8(wo[QW * c:QW * (c + 1), :], SWO),
            "cosE": cosE.astype(npdt),
            "sinE": np.ascontiguousarray(sinE).astype(npdt),
        })
    return in_maps


_NC_CACHE = {}


def get_program():
    if MM not in _NC_CACHE:
        _NC_CACHE[MM] = build_program()
    return _NC_CACHE[MM]


def kernel(x, freqs_cos, freqs_sin, wq, wk, wv, wo):
    nc = get_program()
    in_maps = _host_prep(x, freqs_cos, freqs_sin, wq, wk, wv, wo)
    res = run_bass_kernel_spmd(nc, in_maps, core_ids=list(range(NCORES)))
    acc = np.zeros((S, DIM), np.float64)
    for r in res.results:
        acc += r["out"].astype(np.float64)
    return acc.astype(np.float32).reshape(1, S, DIM)


# revision 107
# speedup vs baseline: 1.0179x; 1.0179x over previous
"""Trainium2 Bass kernel for a GQA attention layer (dense transformer).

Reference computation (B=1, S=2048, DIM=2048, 32 q-heads, 8 kv-heads, hd=64):
    xq = x @ wq; xk = x @ wk; xv = x @ wv
    rope(xq, xk); GQA causal attention; out = attn @ wo

Sharding: tensor-parallel over heads across 8 cores. Core c owns q-heads
4c..4c+3 (wq cols), kv-head c (wk/wv cols), and wo rows 256c..256c+255.
Each core computes a full [S, DIM] partial of the output projection; the
host sums the 8 partials (the TP all-reduce, done at gather time).

Kernel layout strategy (everything "transposed", head_dim on partitions):
  - QT/KT/VT = W.T @ x as compensated-fp8 DoubleRow matmuls: x and wqkv are
    split hi/lo into e4m3 on the host (power-of-two scales SX/SW, folded
    back out through cos/sin for Q/K and the V-copy scalar). Each DR matmul
    contracts a pair of 128-row k-tiles at 0.5 cycles/column; 3 products
    (hi*hi + lo*hi + hi*lo) give better-than-bf16 accuracy at 75% of the
    bf16 PE cost.
  - RoPE pairs are de-interleaved by permuting wq/wk columns on the host so
    the rotation partner sits 16 partitions away (within a 32-partition
    quadrant, so DVE stream_shuffle can swap them); bf16 temporaries keep
    most rope ops in the DVE fast modes.
  - Scores are computed transposed: S^T[k, q] = K^T.T @ Q^T per 128-row
    k-tile; exp on ACT (scale fused); causal mask = upper-tri multiply on
    the single diagonal 128x128 block of each k-tile.
  - P@V is computed as V'.T @ P^T where V' = [V | ones]: the ones column
    makes row 64 of the PSUM accumulator the softmax denominator for free.
  - PV runs two heads behind scores (deep software pipeline) so the
    exp->mask latency of head h hides under PV of head h-2.
  - Normalization: reciprocal (DVE) + partition_broadcast (GPSIMD) + mult,
    then a hi/lo e4m3 split of O (on the idle GPSIMD engine; DVE for the
    last chunk) feeding a compensated-fp8 DoubleRow output projection
    (wo is host-split hi/lo; scales folded out in the PSUM->SBUF copy).
  - WO partials DMA'd bf16 and summed on the host (the TP all-reduce).
"""

import numpy as np
import ml_dtypes

import concourse.bass as bass
import concourse.mybir as mybir
from concourse import bacc
from concourse.tile import TileContext
from concourse.masks import make_identity, make_upper_triangular
from concourse.bass_utils import run_bass_kernel_spmd

# ---------------------------------------------------------------- constants
S = 2048          # sequence length
DIM = 2048        # model dim
NH = 32           # query heads
NKV = 8           # kv heads
HD = 64           # head dim
NCORES = 8
HQ = NH // NCORES          # query heads per core = 4
QW = HQ * HD               # q width per core = 256
KT_S = S // 128            # 16 seq k-tiles
KT_D = DIM // 128          # 16 dim k-tiles
NSC = S // 512             # 4 s-chunks
SCALE = 1.0 / 8.0          # 1/sqrt(64)

# fp8 hi/lo split scales for the projections (powers of two; folded back out
# via cos/sin for Q/K and the V-copy scalar). x absmax ~5, w absmax ~0.11.
SX = 32.0
SW = 1024.0
INV_SXW = 1.0 / (SX * SW)
# fp8 scales for the output projection (O is rescaled at the V-copy; the
# product scale is folded out in the PSUM->SBUF copy of the WO result)
SO = 32.0
SWO = 1024.0
INV_OW = 1.0 / (SO * SWO)

# matmul dtype knob: 'bf16' | 'f32' | 'f32r'
MM = 'bf16'

_SHUF_SWAP16 = list(range(16, 32)) + list(range(16))


def _dtypes():
    if MM == 'bf16':
        return mybir.dt.bfloat16, mybir.dt.bfloat16, ml_dtypes.bfloat16
    if MM == 'f32':
        return mybir.dt.float32, mybir.dt.float32, np.float32
    if MM == 'f32r':
        return mybir.dt.float32, mybir.dt.float32r, np.float32
    raise ValueError(MM)


def _mm_ap(ap, mmdt):
    """View an AP in the matmul dtype (bitcast f32 -> f32r when needed)."""
    if ap.dtype != mmdt:
        return ap.bitcast(mmdt)
    return ap


def build_program():
    """Build the per-core Bass program (same program on all 8 cores).

    Emission is a fine-grained software pipeline: attention beats for chunk
    sc (S^T mega-matmul for head h + PV pairs of head h-1) are merged with
    the projection matmuls of chunk sc+1 and the WO units of chunk sc-1.
    Engines execute in order, so the merge keeps only likely-ready work in
    the PE stream while ACT (exp) and DVE (RoPE/normalize) drain.

    PSUM (8 banks): pjo 3 (projection passes + PV accumulators, shared tag)
    + ps 4 (two [128,1024] score megas) + pw 1 (WO).
    """
    sdt, mmdt, _ = _dtypes()
    f32 = mybir.dt.float32

    nc = bacc.Bacc("TRN2", target_bir_lowering=False, debug=False,
                   num_devices=NCORES)

    f8 = mybir.dt.float8e4
    DR = mybir.MatmulPerfMode.DoubleRow
    xT = nc.dram_tensor("xT8", [DIM, S, 2], f8, kind="ExternalInput")
    wqkv = nc.dram_tensor("wqkv8", [DIM, QW + 2 * HD, 2], f8,
                          kind="ExternalInput")
    wo_s = nc.dram_tensor("wo8", [QW, DIM, 2], f8, kind="ExternalInput")
    cosE = nc.dram_tensor("cosE", [64, S], sdt, kind="ExternalInput")
    sinE = nc.dram_tensor("sinE", [64, S], sdt, kind="ExternalInput")
    out = nc.dram_tensor("out", [S, DIM], sdt, kind="ExternalOutput")

    WQKV = QW + 2 * HD  # 384

    import contextlib
    with TileContext(nc) as tc, contextlib.ExitStack() as ctx:
        const = ctx.enter_context(tc.tile_pool(name="const", bufs=1))
        work = ctx.enter_context(tc.tile_pool(name="work", bufs=2))
        xtp = ctx.enter_context(tc.tile_pool(name="xtp", bufs=9))
        ptp = ctx.enter_context(tc.tile_pool(name="ptp", bufs=26))
        small = ctx.enter_context(tc.tile_pool(name="small", bufs=5))
        osb = ctx.enter_context(tc.tile_pool(name="osb", bufs=4))

        pjo = ctx.enter_context(tc.tile_pool(name="pjo", bufs=3,
                                             space="PSUM"))
        ps = ctx.enter_context(tc.tile_pool(name="ps", bufs=2, space="PSUM"))
        pw = ctx.enter_context(tc.tile_pool(name="pw", bufs=1, space="PSUM"))

        # ----------------------------------------------- persistent SBUF
        w_sb = const.tile([128, KT_D * 2 * WQKV], f8, tag="w_sb")
        wo_sb = const.tile([128, 2 * DIM * 2], f8, tag="wo_sb")
        cos_sb = const.tile([128, S], sdt, tag="cos_sb")
        sin_sb = const.tile([128, S], sdt, tag="sin_sb")
        utri_sb = const.tile([128, 128], sdt, tag="utri_sb")
        ident = const.tile([128, 128], sdt, tag="ident")
        QT = const.tile([64, HQ * S], sdt, tag="QT")
        KVt = const.tile([128, S], sdt, tag="KVt")
        Vp = const.tile([128, KT_S * (HD + 1)], sdt, tag="Vp")
        OTh = const.tile([128, 2 * S], f8, tag="OTh")
        OTl = const.tile([128, 2 * S], f8, tag="OTl")

        # Warm-ups FIRST (minimal dependencies): a throwaway matmul starts
        # the PE p-state ramp clock at t~0 (so the real prologue runs at
        # full clock), and a throwaway exp pulls the ACT table load off the
        # first score tile's critical path.
        warm_src = small.tile([128, 16], sdt, tag="warm_src",
                              name="warm_src")
        warm_ps = pw.tile([16, 16], f32, tag="pw", name="warm")
        warm_sb = small.tile([1, 1], f32, tag="warm_sb", name="warm_sb")
        nc.gpsimd.memset(warm_src[:], 1.0)
        nc.tensor.matmul(warm_ps[:], warm_src[:], warm_src[:],
                         start=True, stop=True)
        nc.scalar.activation(warm_sb[:], warm_ps[0:1, 0:1],
                             mybir.ActivationFunctionType.Exp)

        make_identity(nc, ident[:])
        make_upper_triangular(nc, utri_sb[:], val=1.0)
        nc.gpsimd.memset(Vp[:], 1.0)  # ones columns for denominator

        wo_copy_flip = [0]

        # w_sb layout: [128, kt, hl, WQKV]; xt tiles: [128, kt(4), hl, 512].
        # Projections run as compensated fp8 in DoubleRow mode: each matmul
        # contracts a PAIR of adjacent 128-row k-tiles (one hi/lo plane) at
        # 0.5 cycles/column, and each k-pair needs 3 products
        # (hi*hi + lo*hi + hi*lo) -- 24 matmuls per 16-k-tile pass vs 16
        # full-rate bf16 matmuls, a 25% PE saving at better-than-bf16
        # accuracy.
        wv_ = w_sb[:].rearrange("r (k w t) -> r k w t", k=KT_D, t=2)
        NPAIR = KT_D // 2
        PRODS = ((0, 0), (1, 0), (0, 1))  # (w-plane, x-plane)

        # ---------------------------------------------- thunk generators
        def proj_thunks(sc):
            """Projection of chunk sc, unfused: KV pass over all 8 k-pairs
            (one PSUM slot), then K-rope + V copy/transposes overlapped with
            the two Q passes."""
            s0 = sc * 512
            xts = []
            st = {}

            batches = [4, 4, 2, 2, 2, 2] if sc == 0 else [4, 4, 4, 4]
            starts = [sum(batches[:i]) for i in range(len(batches))]
            pair_slot = {}
            for bi, (b0, bn) in enumerate(zip(starts, batches)):
                for j in range(0, bn, 2):
                    pair_slot[(b0 + j) // 2] = (bi, j, bn, b0)

            def xtv(a):
                bi, j, bn, b0 = pair_slot[a]
                return xts[bi][:].rearrange(
                    "r (k c t) -> r k c t", k=4, t=2)[:, j:j + 2]

            def dma_pair(a):
                bi, j, bn, b0 = pair_slot[a]
                if j == 0:
                    if sc == 0:
                        nc.sync.dma_start(
                            w_sb[:, b0 * 2 * WQKV:(b0 + bn) * 2 * WQKV
                                 ].rearrange("r (k w) -> r k w", k=bn),
                            wqkv[b0 * 128:(b0 + bn) * 128].rearrange(
                                "(k r) w t -> r k (w t)", k=bn))
                    xt4 = xtp.tile([128, 4 * 2 * 512], f8, tag="xt",
                                   name="xt4")
                    nc.sync.dma_start(
                        xt4[:, 0:bn * 1024].rearrange(
                            "r (k c) -> r k c", k=bn),
                        xT[b0 * 128:(b0 + bn) * 128,
                           s0:s0 + 512].rearrange(
                            "(k r) c t -> r k (c t)", k=bn))
                    xts.append(xt4)
                    if sc == 0 and bi == 2:
                        # only chunk 0's 512 columns of cos/sin ride in the
                        # DMA-bound prologue; the rest follows during chunk
                        # 0's attention when the DMA queue is idle
                        # (cos/sin have 64-row periodicity: DMA'd twice)
                        nc.sync.dma_start(cos_sb[0:64, 0:512],
                                          cosE[:, 0:512])
                        nc.sync.dma_start(sin_sb[0:64, 0:512],
                                          sinE[:, 0:512])
                        nc.sync.dma_start(cos_sb[64:128, 0:512],
                                          cosE[:, 0:512])
                        nc.sync.dma_start(sin_sb[64:128, 0:512],
                                          sinE[:, 0:512])

            def kv_mm(a):
                if a == 0:
                    st["pkv"] = pjo.tile([128, 512], f32, tag="pjo",
                                         name="pkv")
                xv = xtv(a)
                for pi, (wp, xp) in enumerate(PRODS):
                    nc.tensor.matmul(
                        st["pkv"][:],
                        wv_[:, 2 * a:2 * a + 2, 256:384, wp],
                        xv[:, :, :, xp],
                        start=(a == 0 and pi == 0),
                        stop=(a == NPAIR - 1 and pi == len(PRODS) - 1),
                        perf_mode=DR)

            def k_rope():
                pkv = st["pkv"]
                shufk = work.tile([64, 512], f32, tag="shufk", name="shufk")
                m1k = work.tile([64, 512], sdt, tag="m1k", name="m1k")
                t2k = work.tile([64, 512], sdt, tag="t2k", name="t2k")
                nc.vector.stream_shuffle(shufk[:], pkv[0:64, :],
                                         _SHUF_SWAP16)
                nc.vector.tensor_mul(m1k[:], pkv[0:64, :],
                                     cos_sb[0:64, s0:s0 + 512])
                nc.vector.tensor_mul(t2k[:], shufk[:],
                                     sin_sb[0:64, s0:s0 + 512])
                nc.vector.tensor_add(KVt[0:64, s0:s0 + 512], m1k[:], t2k[:])

            def v_copy():
                # also folds the fp8 scale back out of V
                nc.vector.tensor_scalar_mul(KVt[64:128, s0:s0 + 512],
                                            st["pkv"][64:128, :],
                                            INV_SXW * SO)

            def v_trans():
                # all 4 transposes of the chunk go into one PSUM tile as a
                # single accumulation group (disjoint columns), then one
                # strided ACT copy drops them into Vp (ACT is idle here)
                pv4 = pw.tile([128, 4 * HD], sdt, tag="pw", name="pv4")
                for j in range(4):
                    kt = 4 * sc + j
                    nc.tensor.matmul(
                        pv4[:, j * HD:(j + 1) * HD],
                        KVt[64:128, kt * 128:(kt + 1) * 128],
                        ident[64:128, 64:128],
                        is_transpose=True, start=(j == 0), stop=(j == 3))
                dst = Vp[:, 4 * sc * (HD + 1):(4 * sc + 4) * (HD + 1)]
                nc.scalar.copy(
                    dst.rearrange("r (k w) -> r k w", k=4)[:, :, 0:HD],
                    pv4[:].rearrange("r (k w) -> r k w", k=4))

            def q_mm(mt, a):
                if mt == 0:
                    dma_pair(a)
                if a == 0:
                    st["pq"] = pjo.tile([128, 512], f32, tag="pjo",
                                        name="pq")
                xv = xtv(a)
                for pi, (wp, xp) in enumerate(PRODS):
                    nc.tensor.matmul(
                        st["pq"][:],
                        wv_[:, 2 * a:2 * a + 2,
                            mt * 128:mt * 128 + 128, wp],
                        xv[:, :, :, xp],
                        start=(a == 0 and pi == 0),
                        stop=(a == NPAIR - 1 and pi == len(PRODS) - 1),
                        perf_mode=DR)

            def q_rope(mt):
                pq = st["pq"]
                shuf = work.tile([128, 512], f32, tag="shuf", name="shuf")
                m1 = work.tile([128, 512], sdt, tag="m1", name="m1")
                t2 = work.tile([128, 512], sdt, tag="t2", name="t2")
                nc.vector.stream_shuffle(shuf[:], pq[:], _SHUF_SWAP16)
                nc.vector.tensor_mul(m1[:], pq[:], cos_sb[:, s0:s0 + 512])
                nc.vector.tensor_mul(t2[:], shuf[:],
                                     sin_sb[:, s0:s0 + 512])
                he = (2 * mt) * S
                ho = (2 * mt + 1) * S
                nc.vector.tensor_add(
                    QT[:, he + s0:he + s0 + 512], m1[0:64, :], t2[0:64, :])
                nc.vector.tensor_add(
                    QT[:, ho + s0:ho + s0 + 512], m1[64:128, :],
                    t2[64:128, :])

            # pass order Q0 -> KV -> Q1: the Q0 rope chain (longest,
            # and first consumed by scores) starts as soon as the last x
            # batch lands, overlapping the KV and Q1 passes on PE; K-rope
            # and the V path overlap Q1.
            th = [lambda a=a: q_mm(0, a) for a in range(NPAIR)]
            th.append(lambda: q_rope(0))
            th += [lambda a=a: kv_mm(a) for a in range(NPAIR)]
            th.append(v_copy)
            th.append(k_rope)
            qth = [lambda a=a: q_mm(1, a) for a in range(NPAIR)]
            qth.insert(2, v_trans)
            th += qth
            th.append(lambda: q_rope(1))
            return th

        def s_thunks(qc, h, tiles):
            """S^T mega matmuls + exp + mask for one head; fills `tiles`."""
            q0 = qc * 512
            hf = h * S
            nkt = 4 * qc + 4
            thunks = []
            for pi in range(nkt // 2):
                def th(pi=pi):
                    kts = (2 * pi, 2 * pi + 1)
                    ps_t = ps.tile([128, 1024], f32, tag="ps", name="ps_t")
                    pt_t = ptp.tile([128, 1024], sdt, tag="pt", name="pt_t")
                    for li, kt in enumerate(kts):
                        dj = kt - 4 * qc
                        qo = 128 * dj if dj >= 0 else 0
                        lo = li * 512
                        nc.tensor.matmul(
                            ps_t[:, lo + qo:lo + 512],
                            _mm_ap(KVt[0:64, kt * 128:(kt + 1) * 128], mmdt),
                            _mm_ap(QT[:, hf + q0 + qo:hf + q0 + 512], mmdt),
                            start=True, stop=True)
                    if 2 * pi + 1 < 4 * qc:
                        nc.scalar.activation(
                            pt_t[:], ps_t[:],
                            mybir.ActivationFunctionType.Exp, scale=SCALE)
                    else:
                        for li, kt in enumerate(kts):
                            dj = kt - 4 * qc
                            qo = 128 * dj if dj >= 0 else 0
                            lo = li * 512
                            nc.scalar.activation(
                                pt_t[:, lo + qo:lo + 512],
                                ps_t[:, lo + qo:lo + 512],
                                mybir.ActivationFunctionType.Exp,
                                scale=SCALE)
                    for li, kt in enumerate(kts):
                        dj = kt - 4 * qc
                        qo = 128 * dj if dj >= 0 else 0
                        lo = li * 512
                        if dj >= 0:
                            nc.vector.tensor_mul(
                                pt_t[:, lo + qo:lo + qo + 128],
                                pt_t[:, lo + qo:lo + qo + 128], utri_sb[:])
                        tiles.append((kt, qo, lo, pt_t))
                thunks.append(th)
            return thunks

        def pv_thunks(qc, h, tiles):
            """PV accumulation pairs + final normalization for one head."""
            q0 = qc * 512
            hp = (h % 2) * 64
            nkt0 = 4 * qc + 4
            state = {}

            def pv_pair(pi):
                if "po" not in state:
                    state["po"] = pjo.tile([HD + 1, 512], f32, tag="pjo",
                                           name="pot")
                po_t = state["po"]
                for kt, qo, lo, pt_t in tiles[2 * pi:2 * pi + 2]:
                    nc.tensor.matmul(
                        po_t[:, qo:512],
                        _mm_ap(Vp[:, kt * (HD + 1):(kt + 1) * (HD + 1)],
                               mmdt),
                        _mm_ap(pt_t[:, lo + qo:lo + 512], mmdt),
                        start=(kt == 0), stop=(kt == nkt0 - 1))
                if 2 * pi + 2 >= nkt0:
                    rc = small.tile([1, 512], f32, tag="rc", name="rc")
                    rb = small.tile([64, 512], f32, tag="rb", name="rb")
                    tn = work.tile([128, 512], sdt, tag="tn",
                                   name="tn")[hp:hp + 64, :]
                    nc.vector.reciprocal(rc[:], po_t[64:65, :])
                    nc.gpsimd.partition_broadcast(rb[:], rc[:])
                    of = (h // 2) * S
                    sl = slice(of + q0, of + q0 + 512)
                    nc.vector.tensor_mul(tn[:], po_t[0:64, :], rb[:])
                    # hi/lo fp8 split of O: on the idle GPSIMD engine for
                    # chunks 0-2; on DVE for the last chunk (shorter tail)
                    eng = nc.vector if qc == NSC - 1 else nc.gpsimd
                    eng.tensor_copy(OTh[hp:hp + 64, sl], tn[:])
                    eng.tensor_sub(OTl[hp:hp + 64, sl], tn[:],
                                   OTh[hp:hp + 64, sl])

            return [lambda pi=pi: pv_pair(pi) for pi in range(nkt0 // 2)]

        def wo_half(qt, np2, half, obs, pool=None, ptag="pw",
                    act_copy=False, dma_mode="row"):
            """One 512-wide n-chunk. dma_mode picks the output-DMA grain:
            "row" = one [128,2048] DMA per q-tile (fewest descriptors, for
            the steady state), "pair" = [128,1024] per np2 pair, "split" =
            [128,512] per chunk (for the very last q-tile so only 512
            columns ride the drain)."""
            pool = pool or pw
            ncn = 2 * np2 + half
            if ncn == 0:
                obs[qt] = osb.tile([128, 2048], sdt, tag="ob", name="ob")
            ob = obs[qt]
            pw_t = pool.tile([128, 512], f32, tag=ptag, name="pw_t")
            qs = slice(qt * 128, qt * 128 + 128)
            ds = slice(ncn * 512, ncn * 512 + 512)
            OThv = OTh[:].rearrange("r (m s) -> r m s", m=2)
            OTlv = OTl[:].rearrange("r (m s) -> r m s", m=2)
            wov = wo_sb[:].rearrange("r (m d t) -> r m d t", m=2, t=2)
            for pi, (o_, w_) in enumerate(((OThv, 0), (OThv, 1), (OTlv, 0))):
                nc.tensor.matmul(
                    pw_t[:], o_[:, :, qs], wov[:, :, ds, w_],
                    start=(pi == 0), stop=(pi == 2), perf_mode=DR)
            if act_copy:
                nc.scalar.mul(ob[:, ncn * 512:ncn * 512 + 512], pw_t[:],
                              INV_OW)
            else:
                nc.vector.tensor_scalar_mul(
                    ob[:, ncn * 512:ncn * 512 + 512], pw_t[:], INV_OW)
            if dma_mode == "split":
                nc.sync.dma_start(
                    out[qt * 128:(qt + 1) * 128, ncn * 512:ncn * 512 + 512],
                    ob[:, ncn * 512:ncn * 512 + 512])
            elif dma_mode == "pair" and half == 1:
                nc.sync.dma_start(
                    out[qt * 128:(qt + 1) * 128,
                        np2 * 1024:np2 * 1024 + 1024],
                    ob[:, np2 * 1024:np2 * 1024 + 1024])
            elif dma_mode == "row" and ncn == 3:
                nc.sync.dma_start(out[qt * 128:(qt + 1) * 128, :], ob[:])
            if ncn == 3:
                del obs[qt]
                wo_copy_flip[0] += 1
        wo_obs = {}

        # ------------------------------------- merged emission schedule
        def merge(primary, *others):
            """Emit primary thunks; proportionally interleave the others."""
            counters = [0.0] * len(others)
            n = max(1, len(primary))
            for beat in primary:
                for j, lst in enumerate(others):
                    counters[j] += len(lst) / n
                    while counters[j] >= 1.0 and lst:
                        lst.pop(0)()
                        counters[j] -= 1.0
                for th in beat:
                    th()
            for lst in others:
                while lst:
                    lst.pop(0)()

        for th in proj_thunks(0):                   # prologue
            th()
        # columns 512:S of cos/sin, needed from chunk 1's rope onward,
        # stream in while the DMA queue idles during chunk 0's attention
        nc.sync.dma_start(cos_sb[0:64, 512:S], cosE[:, 512:S])
        nc.sync.dma_start(sin_sb[0:64, 512:S], sinE[:, 512:S])
        nc.sync.dma_start(cos_sb[64:128, 512:S], cosE[:, 512:S])
        nc.sync.dma_start(sin_sb[64:128, 512:S], sinE[:, 512:S])

        hist = []                        # (qc, h, tiles) awaiting PV
        for sc in range(NSC):
            if sc == 1:
                nc.sync.dma_start(
                    wo_sb[:, 0:2 * DIM],
                    wo_s[0:128].rearrange("r d t -> r (d t)"))
                nc.sync.dma_start(
                    wo_sb[:, 2 * DIM:4 * DIM],
                    wo_s[128:256].rearrange("r d t -> r (d t)"))
            pstream = proj_thunks(sc + 1) if sc + 1 < NSC else []
            wostream = ([lambda qt=qt, np2=np2, half=half, ac=False:
                         wo_half(qt, np2, half, wo_obs, act_copy=ac)
                         for qt in range(4 * (sc - 1), 4 * (sc - 1) + 4)
                         for np2 in range(2)
                         for half in range(2)] if sc >= 1 else [])
            for h in range(HQ):
                tiles = []
                sth = s_thunks(sc, h, tiles)
                pth = pv_thunks(*hist[-2]) if len(hist) >= 2 else []
                beats = []
                for i in range(max(len(sth), len(pth))):
                    beat = []
                    if i < len(pth):
                        beat.append(pth[i])
                    if i < len(sth):
                        beat.append(sth[i])
                    beats.append(beat)
                # WO of sc-1 needs PV(sc-1, 3) done: that PV is head 0 here
                if h == 0:
                    ptake = max(1, len(pstream) // HQ) if pstream else 0
                    merge(beats, pstream[:ptake])
                    pstream = pstream[ptake:]
                else:
                    # on the last chunk hold 4 WO thunks back: they fill the
                    # PE while the final head's normalize chain drains
                    hold = 3 if sc == NSC - 1 else 0
                    avail = max(0, len(wostream) - hold)
                    ptake = (len(pstream) // (HQ - h)) if pstream else 0
                    wtake = (avail // (HQ - h)) if avail and h >= 3 else 0
                    merge(beats, pstream[:ptake], wostream[:wtake])
                    pstream = pstream[ptake:]
                    wostream = wostream[wtake:]
                hist.append((sc, h, tiles))
            if sc < NSC - 1:
                merge([], pstream, wostream)

        # epilogue: PV of the last head (+ the held-back chunk-2 WO units
        # covering its normalize), then WO of chunk 3; the score pool's
        # banks are free now, so WO rotates through those too
        for th in pv_thunks(*hist[-2]):
            th()
        for i, th in enumerate(wostream):
            th(ac=True)
            if i == 1:
                for t2 in pv_thunks(*hist[-1]):
                    t2()
        epi = 0
        pools = [(pw, "pw"), (ps, "ps"), (pjo, "pjo"),
                 (ps, "ps"), (pjo, "pjo"), (pjo, "pjo")]
        for qt in (12, 13, 14, 15):
            for np2 in range(2):
                for half in range(2):
                    pool, ptag = pools[epi % len(pools)]
                    wo_half(qt, np2, half, wo_obs, pool=pool, ptag=ptag,
                            act_copy=(epi % 2 == 0),
                            dma_mode="pair")
                    epi += 1

    nc.compile()
    return nc


# ------------------------------------------------------------- host side
def _pair_perm64():
    """Column permutation putting the RoPE partner 16 partitions away."""
    return np.array([2 * (16 * (j // 32) + (j % 16)) + ((j % 32) // 16)
                     for j in range(64)])


def _host_prep(x, freqs_cos, freqs_sin, wq, wk, wv, wo):
    _, _, npdt = _dtypes()
    x = np.asarray(x, np.float32)
    fc = np.asarray(freqs_cos, np.float32)
    fs = np.asarray(freqs_sin, np.float32)
    wq = np.asarray(wq, np.float32)
    wk = np.asarray(wk, np.float32)
    wv = np.asarray(wv, np.float32)
    wo = np.asarray(wo, np.float32)

    perm = _pair_perm64()
    f8np = ml_dtypes.float8_e4m3

    def split8(a, s):
        hi = np.asarray(a * s, f8np)
        lo = np.asarray(a * s - hi.astype(np.float32), f8np)
        return np.ascontiguousarray(np.stack([hi, lo], axis=-1))

    xT8 = split8(x[0].T, SX)                 # [DIM, 2, S] fp8

    p = np.arange(64)
    pair = 16 * ((p % 64) // 32) + (p % 16)
    sign = np.where((p % 32) < 16, -1.0, 1.0).astype(np.float32)
    # cos/sin also fold the fp8 projection scale back out of Q and K
    cosE = np.ascontiguousarray(fc[:, pair].T) * INV_SXW        # [64, S]
    sinE = (np.ascontiguousarray(fs[:, pair].T) * sign[:, None]) * INV_SXW

    in_maps = []
    for c in range(NCORES):
        qcols = np.concatenate(
            [wq[:, (4 * c + i) * 64 + perm] for i in range(HQ)], axis=1)
        kcols = wk[:, c * 64 + perm]
        vcols = wv[:, c * 64:(c + 1) * 64]
        wqkv_c = np.concatenate([qcols, kcols, vcols], axis=1)
        in_maps.append({
            "xT8": xT8,
            "wqkv8": split8(wqkv_c, SW),
            "wo8": split8(wo[QW * c:QW * (c + 1), :], SWO),
            "cosE": cosE.astype(npdt),
            "sinE": np.ascontiguousarray(sinE).astype(npdt),
        })
    return in_maps


_NC_CACHE = {}


def get_program():
    if MM not in _NC_CACHE:
        _NC_CACHE[MM] = build_program()
    return _NC_CACHE[MM]


def kernel(x, freqs_cos, freqs_sin, wq, wk, wv, wo):
    nc = get_program()
    in_maps = _host_prep(x, freqs_cos, freqs_sin, wq, wk, wv, wo)
    res = run_bass_kernel_spmd(nc, in_maps, core_ids=list(range(NCORES)))
    acc = np.zeros((S, DIM), np.float64)
    for r in res.results:
        acc += r["out"].astype(np.float64)
    return acc.astype(np.float32).reshape(1, S, DIM)
